# revision 26
# baseline (speedup 1.0000x reference)
"""Trainium2 Bass kernel for CapsuleLayer dynamic routing (fp16 rewrite).

Math (reference):
    u_hat[b,i,j,e] = sum_d inputs[b,i,d] * kernel[i,j,d,e]
    3 routing iterations over shared bias[i,j] (softmax over j),
    s[b,j,e] = sum_i c[i,j] u_hat[b,i,j,e]; outputs = squash(s)
    bias += sum_{b,e} u_hat * outputs

Strategy: shard i (in_caps=1152 -> 144/core) across 8 cores; u_hat never
materialized.  All data SBUF-resident in fp16 (halves DMA + SBUF traffic,
~5e-4 elementwise error, and unlocks the DVE 2x_1p perf mode for the two
big 2.36M-element elementwise passes).  K is stored with (e,j) innermost
so the c[i,j] broadcast in the scale op lands on a middle dim, keeping
the output packed (2x mode requires unit-stride last dims).

Per routing iteration:
    G = X^T @ O per 128-row chunk (PE, psum) -> fp16 copy (Act)
    P = K (*) G              (DVE fp16 2x)
    d-sum on PE: 16 chunks' P accumulate through 0/1 stationary masks
    into Q[(c,i8),(e,j)] (one psum tile) -- this replaces what would be
    a 2.36M-element DVE e-reduce (tensor_reduce has no fp16 fast mode)
    with a single 147K-element pass:
    e-reduce of Q (DVE, one 1k-elem/partition pass) -> incr_t[(c,i8), j]
    replicate to the (i8,d)-partition layout via PE broadcast matmuls
    (tail 2 chunks take the e-reduce + sel8 path directly).
    NOTE: matmuls with non-32-aligned tile_position offsets fault on HW
    (ran in CoreSim, died at runtime on device) -- hence full [128,128]
    stationary masks instead of 32-row tiles.
    bias += incr; softmax over j (no max-sub: logits O(10))
    cK = c (x) K             (DVE fp16 2x, 3 ops)
    s = X @ cK               (PE, 18-chunk psum accumulation)
    AllReduce(s) in fp16; squash -> O
Final iteration: ReduceScatter, each core squashes + emits its 8 batches.

Measured: 155,995 ns HW (repeat-delta), rel err 2.3e-3 (gate 2e-2).
Baseline (f32r, per-chunk DMAs, all-DVE elementwise): 214,278 ns.
"""

import sys

import numpy as np

if "/opt/trn_rl_repo" not in sys.path:
    sys.path.insert(0, "/opt/trn_rl_repo")

B, I, D, J, E = 64, 1152, 16, 32, 32
N_CORES = 8
I_LOC = I // N_CORES            # 144
ID = I_LOC * D                  # 2304
NCHUNK = ID // 128              # 18
NMAIN = 16                      # d-sum-first chunks (full-stationary masks, no tile_position)
ISUB = 128 // D                 # 8 distinct i per 128-row chunk
JE = J * E                      # 1024
BSH = B // N_CORES              # 8 output batches per core
EPS = 1e-7
ROUTING_STEPS = 2               # routing iters after the uniform-c step
KSC = 6                         # K-chunks per K-scale DVE op

_CACHE = {}
AR_F32 = False  # fp16 collectives halve AR payload
SPLIT_DMA = False


def _build_nc(repeat=1, comm=True, debug_taps=False, ar_f32=False):
    import concourse.mybir as mybir
    import concourse.tile as tile
    from concourse import bacc

    f32 = mybir.dt.float32
    f16 = mybir.dt.float16
    AX = mybir.AxisListType
    OP = mybir.AluOpType
    AF = mybir.ActivationFunctionType

    nc = bacc.Bacc("TRN2", target_bir_lowering=False, debug=False,
                   num_devices=N_CORES)
    x_d = nc.dram_tensor("x", [B, ID], f16, kind="ExternalInput")
    xt_d = nc.dram_tensor("xt", [ID, B], f16, kind="ExternalInput")
    k_d = nc.dram_tensor("kk", [ID, JE], f16, kind="ExternalInput")
    # packed constants: [sel8(128) | selfull(16*128) | bmask(16*128)]
    cst_d = nc.dram_tensor("cst", [128, 4224], f16, kind="ExternalInput")
    out_d = nc.dram_tensor("out", [BSH, JE], f32, kind="ExternalOutput")
    fAR = f32 if ar_f32 else f16
    arin_d = nc.dram_tensor("ar_in", [B, JE], fAR)
    arout_d = nc.dram_tensor("ar_out", [B, JE], fAR, addr_space="Shared")
    rsout_d = nc.dram_tensor("rs_out", [BSH, JE], fAR)
    if debug_taps:
        dbg_s0 = nc.dram_tensor("dbg_s0", [B, JE], fAR, kind="ExternalOutput")
        dbg_orr = nc.dram_tensor("dbg_orr", [B, JE], f16,
                                 kind="ExternalOutput")
        dbg_incr = nc.dram_tensor("dbg_incr", [128, NCHUNK * J], f32,
                                  kind="ExternalOutput")
        dbg_crep = nc.dram_tensor("dbg_crep", [128, NCHUNK * J], f16,
                                  kind="ExternalOutput")
        dbg_s1 = nc.dram_tensor("dbg_s1", [B, JE], f16, kind="ExternalOutput")
    RG = [list(range(N_CORES))]

    with tile.TileContext(nc) as tc:
        with (
            tc.tile_pool(name="big", bufs=1) as big,
            tc.tile_pool(name="work", bufs=2) as work,
            tc.tile_pool(name="once", bufs=1) as once,
            tc.tile_pool(name="small", bufs=2) as small,
            tc.tile_pool(name="gps", bufs=2, space="PSUM") as gps,
            tc.tile_pool(name="sps", bufs=1, space="PSUM") as sps,
            tc.tile_pool(name="qps", bufs=1, space="PSUM") as qps,
        ):
            # ---- resident inputs; few big DMAs (SP issuance is ~600ns each,
            # so 38 small DMAs would serialize ~25us of issue time)
            ksb = big.tile([128, NCHUNK, JE], f16)
            xtsb = big.tile([128, NCHUNK, B], f16)
            if SPLIT_DMA:
                for c in range(NCHUNK):
                    nc.sync.dma_start(ksb[:, c, :],
                                      k_d[c * 128:(c + 1) * 128, :])
                    nc.sync.dma_start(xtsb[:, c, :],
                                      xt_d[c * 128:(c + 1) * 128, :])
            else:
                for g in range(3):
                    nc.sync.dma_start(
                        ksb[:, g * 6:(g + 1) * 6, :],
                        k_d[:].rearrange("(c p) n -> p c n",
                                         p=128)[:, g * 6:(g + 1) * 6, :])
                nc.sync.dma_start(
                    xtsb[:], xt_d[:].rearrange("(c p) n -> p c n", p=128))
            xsb = big.tile([B, ID], f16)
            nc.sync.dma_start(xsb[:], x_d[:])
            cst = big.tile([128, 4224], f16)
            nc.sync.dma_start(cst[:], cst_d[:])
            sel8 = cst[:, 0:128]
            selfull = cst[:, 128:2176].rearrange("p (m q) -> p m q", q=128)
            bmask = cst[:, 2176:4224].rearrange("p (m q) -> p m q", q=128)

            epsb = big.tile([B, 1], f32)
            nc.vector.memset(epsb[:], EPS)

            bias = big.tile([128, NCHUNK, J], f32)
            crep = big.tile([128, NCHUNK, J], f16)
            s_full = big.tile([B, JE], fAR)
            orr = big.tile([B, JE], f16)

            def emit_squash(alpha, nb, s_in, final):
                # squash(alpha * s_in) over e; layout [(b), (e j)]
                sq = once.tile([B, JE], f32, tag="sq")
                nc.scalar.activation(sq[:nb, :], s_in[:nb, :], AF.Square,
                                     scale=alpha)
                n2 = small.tile([B, J], f32, tag="n2")
                nc.vector.tensor_reduce(
                    n2[:nb, :],
                    sq[:nb, :].rearrange("b (e j) -> b j e", j=J),
                    axis=AX.X, op=OP.add)
                d2s = small.tile([B, J], f32, tag="d2s")
                nc.scalar.activation(d2s[:nb, :], n2[:nb, :], AF.Sqrt,
                                     bias=epsb[:nb, :])
                d1 = small.tile([B, J], f32, tag="d1")
                nc.vector.tensor_scalar_add(d1[:nb, :], n2[:nb, :], 1.0)
                den = small.tile([B, J], f32, tag="den")
                nc.vector.tensor_tensor(den[:nb, :], d1[:nb, :], d2s[:nb, :],
                                        op=OP.mult)
                rcp = small.tile([B, J], f32, tag="rcp")
                nc.vector.reciprocal(rcp[:nb, :], den[:nb, :])
                if alpha != 1.0:
                    n2s = small.tile([B, J], f32, tag="n2s")
                    nc.vector.tensor_scalar_mul(n2s[:nb, :], n2[:nb, :],
                                                alpha)
                else:
                    n2s = n2
                facf = small.tile([B, J], f32, tag="facf")
                nc.vector.tensor_tensor(facf[:nb, :], n2s[:nb, :],
                                        rcp[:nb, :], op=OP.mult)
                fb = facf[:nb, None, :].broadcast_to([nb, E, J])
                s3 = s_in[:nb, :].rearrange("b (e j) -> b e j", j=J)
                if final:
                    osb = once.tile([BSH, JE], f32, tag="osb")
                    with nc.allow_low_precision("fp16 squash"):
                        nc.vector.tensor_tensor(
                            osb[:].rearrange("b (j e) -> b e j", e=E),
                            s3, fb, op=OP.mult)
                    nc.sync.dma_start(out_d[:], osb[:])
                else:
                    with nc.allow_low_precision("fp16 squash"):
                        nc.vector.tensor_tensor(
                            orr[:nb, :].rearrange("b (e j) -> b e j", j=J),
                            s3, fb, op=OP.mult)

            def emit_allreduce(s_ps):
                s_sb = once.tile([B, JE], fAR, tag="s_sb")
                nc.scalar.activation(s_sb[:], s_ps[:], AF.Copy)
                nc.sync.dma_start(arin_d[:], s_sb[:])
                if comm:
                    nc.gpsimd.collective_compute(
                        "AllReduce", OP.add, replica_groups=RG,
                        ins=[arin_d[:]], outs=[arout_d[:]])
                    nc.sync.dma_start(s_full[:], arout_d[:])
                else:
                    nc.sync.dma_start(s_full[:], arin_d[:])

            def emit_s_matmul(rhs_of_chunk):
                s_ps = sps.tile([B, JE], f32, tag="s")
                for c in range(NCHUNK):
                    rhs = rhs_of_chunk(c)
                    for h in range(2):
                        nc.tensor.matmul(
                            s_ps[:, h * 512:(h + 1) * 512],
                            xtsb[:, c, :],
                            rhs[:, h * 512:(h + 1) * 512],
                            start=(c == 0), stop=(c == NCHUNK - 1))
                return s_ps

            for _rep in range(repeat):
                # ---- phase 0: s0 = X @ K (uniform c folded via alpha=1/J)
                s_ps = emit_s_matmul(lambda c: ksb[:, c, :])
                emit_allreduce(s_ps)
                emit_squash(1.0 / J, B, s_full, final=False)
                if debug_taps:
                    nc.sync.dma_start(dbg_s0[:], s_full[:])
                    nc.sync.dma_start(dbg_orr[:], orr[:])

                for r in range(ROUTING_STEPS):
                    # ---- increments.  Per chunk: G = X^T O (PE, psum),
                    # fp16 copy (Act), P = K*G (DVE 2x), then d-sum on PE.
                    if NMAIN:
                        qt = qps.tile([128, JE], f32, tag="qt")
                    incr_ps = sps.tile([128, NCHUNK * J], f32, tag="s")
                    for c in range(NCHUNK):
                        g_ps = gps.tile([128, JE], f32, tag="g")
                        for h in range(2):
                            nc.tensor.matmul(
                                g_ps[:, h * 512:(h + 1) * 512],
                                xsb[:, c * 128:(c + 1) * 128],
                                orr[:, h * 512:(h + 1) * 512],
                                start=True, stop=True)
                        g_sb = work.tile([128, JE], f16, tag="gsb")
                        nc.scalar.activation(g_sb[:], g_ps[:], AF.Copy)
                        psup = work.tile([128, JE], f16, tag="p")
                        nc.vector.tensor_tensor(psup[:], ksb[:, c, :],
                                                g_sb[:], op=OP.mult)
                        if c < NMAIN:
                            for h in range(2):
                                nc.tensor.matmul(
                                    qt[:, h * 512:(h + 1) * 512],
                                    selfull[:, c, :],
                                    psup[:, h * 512:(h + 1) * 512],
                                    start=(c == 0), stop=(c == NMAIN - 1))
                        else:
                            per = small.tile([128, J], f16, tag="per")
                            with nc.allow_low_precision("fp16 incr"):
                                nc.vector.tensor_reduce(
                                    per[:],
                                    psup[:].rearrange("p (e j) -> p j e",
                                                      j=J),
                                    axis=AX.X, op=OP.add)
                            nc.tensor.matmul(
                                incr_ps[:, c * J:(c + 1) * J], sel8[:],
                                per[:], start=True, stop=True)
                    if NMAIN:
                        incr_t = once.tile([128, J], f16, tag="it")
                        with nc.allow_low_precision("fp16 incr"):
                            nc.vector.tensor_reduce(
                                incr_t[:],
                                qt[:].rearrange("p (e j) -> p j e", j=J),
                                axis=AX.X, op=OP.add)
                        for c in range(NMAIN):
                            nc.tensor.matmul(
                                incr_ps[:, c * J:(c + 1) * J],
                                bmask[:, c, :], incr_t[:],
                                start=True, stop=True)
                    bflat = bias[:].rearrange("p c j -> p (c j)")
                    if debug_taps and r == 0:
                        dbg_i = once.tile([128, NCHUNK * J], f32, tag="dbgi")
                        nc.scalar.activation(dbg_i[:], incr_ps[:], AF.Copy)
                        nc.sync.dma_start(dbg_incr[:], dbg_i[:])
                    if r == 0:
                        nc.scalar.activation(bflat, incr_ps[:], AF.Copy)
                    else:
                        nc.vector.tensor_tensor(bflat, bflat, incr_ps[:],
                                                op=OP.add)
                    # ---- softmax over j (skip max-sub; logits are O(10))
                    exe = once.tile([128, NCHUNK, J], f32, tag="exe")
                    nc.scalar.activation(exe[:], bias[:], AF.Exp)
                    sm = small.tile([128, NCHUNK], f32, tag="sm")
                    nc.vector.tensor_reduce(sm[:], exe[:], axis=AX.X,
                                            op=OP.add)
                    rc = small.tile([128, NCHUNK], f32, tag="rc")
                    nc.vector.reciprocal(rc[:], sm[:])
                    with nc.allow_low_precision("fp16 c"):
                        nc.vector.tensor_tensor(
                            crep[:], exe[:],
                            rc[:, :, None].broadcast_to([128, NCHUNK, J]),
                            op=OP.mult)

                    # ---- s_{r+1} = X @ (c (x) K); fp16 2x scale in 3 ops
                    kps = {}

                    def scaled_k(c, _kps=kps):
                        sc, cc = divmod(c, KSC)
                        if sc not in _kps:
                            kp = work.tile([128, KSC, JE], f16, tag="kp")
                            nc.vector.tensor_tensor(
                                kp[:].rearrange("p c (e j) -> p c e j", j=J),
                                ksb[:, sc * KSC:(sc + 1) * KSC, :].rearrange(
                                    "p c (e j) -> p c e j", j=J),
                                crep[:, sc * KSC:(sc + 1) * KSC, None, :]
                                .broadcast_to([128, KSC, E, J]),
                                op=OP.mult)
                            _kps[sc] = kp
                        return _kps[sc][:, cc, :]
                    s_ps = emit_s_matmul(scaled_k)
                    if debug_taps and r == 0:
                        nc.sync.dma_start(
                            dbg_crep[:],
                            crep[:].rearrange("p c j -> p (c j)"))
                        dbg_s1t = once.tile([B, JE], f16, tag="dbgs1")
                        nc.scalar.activation(dbg_s1t[:], s_ps[:], AF.Copy)
                        nc.sync.dma_start(dbg_s1[:], dbg_s1t[:])

                    final = (r == ROUTING_STEPS - 1)
                    if final and comm:
                        # ReduceScatter: core c gets batches c*8..(c+1)*8
                        s_sb = once.tile([B, JE], fAR, tag="s_sb")
                        nc.scalar.activation(s_sb[:], s_ps[:], AF.Copy)
                        nc.sync.dma_start(arin_d[:], s_sb[:])
                        nc.gpsimd.collective_compute(
                            "ReduceScatter", OP.add, replica_groups=RG,
                            ins=[arin_d[:]], outs=[rsout_d[:]])
                        s_sh = once.tile([BSH, JE], fAR, tag="s_sh")
                        nc.sync.dma_start(s_sh[:], rsout_d[:])
                        emit_squash(1.0, BSH, s_sh, final=True)
                    else:
                        emit_allreduce(s_ps)
                        emit_squash(1.0, B, s_full, final=False)
    nc.compile()
    return nc


def _shard_inputs(inputs, kern):
    """Build the 8 per-core input maps (numpy preprocessing, fp16)."""
    # tail path: sel8 d-sums within each 16-row i-block and replicates
    sel8 = np.zeros((128, 128), dtype=np.float16)
    for i8 in range(ISUB):
        sel8[i8 * D:(i8 + 1) * D, i8 * D:(i8 + 1) * D] = 1.0
    # d-sum stationaries: selfull[(i8,d), c, q] = 1 iff q == 8c + i8
    # (chunk c's d-sum lands on Qt rows 8c..8c+7; accumulated over chunks)
    selfull = np.zeros((128, 16, 128), dtype=np.float16)
    # broadcast stationaries: bmask[8c+i8, c, (i8,d)] = 1 replicates
    # incr_t row 8c+i8 across the d-partitions of chunk c's crep block
    bmask = np.zeros((128, 16, 128), dtype=np.float16)
    for c in range(16):
        for i8 in range(ISUB):
            for d in range(D):
                selfull[i8 * D + d, c, 8 * c + i8] = 1.0
                bmask[8 * c + i8, c, i8 * D + d] = 1.0
    cst = np.concatenate(
        [sel8, selfull.reshape(128, 2048), bmask.reshape(128, 2048)], axis=1)
    cst = np.ascontiguousarray(cst, dtype=np.float16)

    in_maps = []
    for c in range(N_CORES):
        lo, hi = c * I_LOC, (c + 1) * I_LOC
        x = np.ascontiguousarray(
            inputs[:, lo:hi, :].reshape(B, ID), dtype=np.float16)
        xt = np.ascontiguousarray(x.T)
        # K with (e, j) innermost: [(i,d), (e,j)]
        kk = np.ascontiguousarray(
            kern[lo:hi].transpose(0, 2, 3, 1).reshape(ID, JE),
            dtype=np.float16)
        in_maps.append({"x": x, "xt": xt, "kk": kk, "cst": cst})
    return in_maps


def kernel(inputs, kernel):
    import time

    from concourse.bass_utils import run_bass_kernel_spmd

    in_maps = _shard_inputs(np.asarray(inputs), np.asarray(kernel))
    last_err = None
    for attempt in range(3):
        try:
            if "nc" not in _CACHE:
                _CACHE["nc"] = _build_nc(repeat=1, ar_f32=AR_F32)
            res = run_bass_kernel_spmd(_CACHE["nc"], in_maps,
                                       list(range(N_CORES)))
            out = np.concatenate(
                [res.results[c]["out"] for c in range(N_CORES)], axis=0)
            return out.reshape(B, J, E).astype(np.float32)
        except Exception as e:  # transient NRT/device hiccups
            last_err = e
            _CACHE.pop("nc", None)
            try:
                import jax
                jax.clear_caches()
            except Exception:
                pass
            time.sleep(2.0 * (attempt + 1))
    raise last_err
